# revision 6
# baseline (speedup 1.0000x reference)
"""CategorySpecificLinear Trainium2 kernel.

out[t] = x[t] @ weight[category_id[t]] + bias[category_id[t]]

Strategy: expert-parallel over the 8 categories (C == n_cores == 8) with a
fixed device capacity of CAP=512 tokens per core. Host routes tokens by
category; the few tokens beyond 512 in an over-subscribed category (counts
are ~512 +/- 25 for T=4096 uniform tokens) are computed on the host during
the unshard step, so the NEFF shape is static.

All device traffic is fp16 (tolerance is 2e-2; fp16 in/out measures ~4e-4):
    xT  [D=1024, 512]  tokens of category c, transposed, zero-padded
    w   [D, O]         weight[c]
    out [512, O]       fp16; bias (+ fp32 cast) is folded into the host
                       scatter -- a vectorized add during unsharding.

Compute is x-stationary: psum[m,n] (+)= x[k,m].T @ w[k,n] over k, with
m = 4 token-tiles of 128 and n = 2 O-halves of 512 -- exactly the 8 fp32
PSUM banks. MM order is phase1 k=0..3 for all (m,n) (gated only on the
k-major DMA stream), then per-m tails k=4..7, so the 8 psum groups COMPLETE
staggered ~2 us apart and the psum->sbuf cast + store of group i overlaps
the matmuls of groups i+1.. instead of stacking after the last MM. Each
LDWEIGHTS x[k,m] is shared by the n-pair of matmuls.

Loads stream k-major in 128 KB pieces (x[k], w[k] in halves) round-robined
over both HWDGE queues (the pair saturates the ~358 GB/s per-core HBM
limit); casts split DVE/ACT; stores split Sync/GpSimd queues.

Per-core HBM traffic ~4.2 MB; PE stream is 64 matmuls @ N=512 fp16.
"""

import contextlib
import ctypes
import os
import sys
import types

import numpy as np

sys.path.insert(0, "/opt/trn_rl_repo")


def _ensure_ntff_hook():
    """Provide antenv.axon_hooks if the image lacks it.

    concourse.bass_utils imports antenv.axon_hooks.get_axon_ntff_profile_hook
    when trace=True under axon; some agent images don't ship that module, in
    which case the boot's NTFF hook registration silently degrades and the
    import in bass_utils crashes. Recreate the slim ctypes hook here
    (mirrors trn_agent_boot.trn_boot._ntff_profile_via_ctypes).
    """
    try:
        import antenv.axon_hooks  # noqa: F401

        return
    except ImportError:
        pass

    so_path = "/opt/axon/libaxon_pjrt.so"
    hook = None
    if os.path.exists(so_path):
        lib = ctypes.CDLL(so_path)
        if hasattr(lib, "axon_start_nrt_profile"):
            lib.axon_start_nrt_profile.argtypes = [
                ctypes.POINTER(ctypes.c_int64),
                ctypes.c_size_t,
            ]
            lib.axon_start_nrt_profile.restype = ctypes.c_int64
            lib.axon_stop_nrt_profile.argtypes = [ctypes.c_char_p]
            lib.axon_stop_nrt_profile.restype = ctypes.c_int64

            @contextlib.contextmanager
            def hook(output_dir, device_ids):
                import jax

                jax.devices()
                if device_ids:
                    ids = (ctypes.c_int64 * len(device_ids))(*device_ids)
                    rc = lib.axon_start_nrt_profile(ids, len(device_ids))
                else:
                    rc = lib.axon_start_nrt_profile(None, 0)
                if rc != 0:
                    raise RuntimeError(f"axon_start_nrt_profile rc={rc}")
                try:
                    yield
                finally:
                    n = lib.axon_stop_nrt_profile(str(output_dir).encode())
                    if n <= 0:
                        print(
                            f"ntff profile: rc={n} writing {output_dir}",
                            file=sys.stderr,
                        )

    mod = types.ModuleType("antenv.axon_hooks")
    _state = {"hook": hook}
    mod.set_axon_ntff_profile_hook = lambda h: _state.__setitem__("hook", h)
    mod.get_axon_ntff_profile_hook = lambda: _state["hook"]
    sys.modules["antenv.axon_hooks"] = mod
    try:
        import antenv

        antenv.axon_hooks = mod
    except ImportError:
        pass


_ensure_ntff_hook()

import concourse.bass as bass
import concourse.bacc as bacc_mod
import concourse.mybir as mybir
import concourse.tile as tile
from concourse.bass import ts
from concourse.bass_utils import run_bass_kernel_spmd

N_CORES = 8
P = 128
CAP = 512  # device tokens per core
D = 1024
O = 1024
KO = D // P  # 8 contraction slices
MO = CAP // P  # 4 token tiles
NT = 512  # O-half (one fp32 PSUM bank)
NO = O // NT  # 2

_nc_cache = {}
LAST_RESULTS = None  # BassKernelResults of the most recent run (for test.py)


def _build_nc():
    f16 = mybir.dt.float16
    f32 = mybir.dt.float32

    nc = bacc_mod.Bacc()
    xT = nc.dram_tensor("xT", [D, CAP], f16, kind="ExternalInput")
    w = nc.dram_tensor("w", [D, O], f16, kind="ExternalInput")
    out = nc.dram_tensor("out", [CAP, O], f16, kind="ExternalOutput")

    xT_t = xT[:, :].rearrange("(ko p) t -> p ko t", p=P)
    w_t = w[:, :].rearrange("(ko p) o -> p ko o", p=P)

    with tile.TileContext(nc) as tc:
        with (
            tc.tile_pool(name="resident", bufs=1) as rpool,
            tc.tile_pool(name="psum", bufs=8, space="PSUM") as psum_pool,
            tc.tile_pool(name="obuf", bufs=8) as opool,
        ):
            # HAM warm-up: dummy matmuls from when the engine frees (~7 us)
            # until the first k-slice lands (~9.3 us) pull the 3.4 us HAM
            # busy-window forward so the real stream runs at 2.4 GHz sooner.
            # The warm psum tile is the first allocation of the 8-buf "ps"
            # ring; its bank is recycled for the last psum group (warm-up is
            # long done by that group's first MM).
            warm_sb = rpool.tile([P, P], f16, tag="warm")
            nc.vector.memset(warm_sb[:], 0.0)
            warm_ps = psum_pool.tile([P, NT], f32, tag="ps", name="warm_ps")
            for _ in range(16):
                nc.tensor.matmul(
                    warm_ps[:, :P],
                    lhsT=warm_sb[:],
                    rhs=warm_sb[:],
                    start=True,
                    stop=True,
                )

            # Loads on THREE queues (a single queue tops out ~120-200 GB/s
            # depending on line size; three together reach the ~320 GB/s
            # per-core HBM rate). x[k] singles on Sync; w[k] kept as full
            # 256 KB pieces (2 KB/partition lines run ~190 GB/s vs ~120 for
            # 1 KB halves) parity-split across Scalar/GpSimd so consecutive
            # k-slices land ~0.7 us apart. Only w[0] is halved across both
            # w-queues to start the PE ~0.4 us earlier.
            x_sb, w_sb = [], []
            for k in range(KO):
                xt = rpool.tile([P, CAP], f16, tag=f"x{k}")
                wt = rpool.tile([P, O], f16, tag=f"w{k}")
                nc.sync.dma_start(xt[:], xT_t[:, k, :])
                if k == 0:
                    nc.scalar.dma_start(wt[:, :NT], w_t[:, k, :NT])
                    nc.gpsimd.dma_start(wt[:, NT:], w_t[:, k, NT:])
                else:
                    eng = nc.gpsimd if k % 2 == 1 else nc.scalar
                    eng.dma_start(wt[:], w_t[:, k, :])
                x_sb.append(xt)
                w_sb.append(wt)

            pss = {
                (m, n): psum_pool.tile([P, NT], f32, tag="ps", name=f"ps{m}_{n}")
                for m in range(MO)
                for n in range(NO)
            }

            # Phase 1: k=0..3 for every (m,n) -- rides the DMA stream.
            # n-outer so the n=0 matmuls never wait for the w-hi piece
            # (which lands on the slower-latency GpSimd queue).
            for k in range(KO // 2):
                for n in range(NO):
                    for m in range(MO):
                        nc.tensor.matmul(
                            pss[(m, n)][:],
                            lhsT=x_sb[k][:, ts(m, P)],
                            rhs=w_sb[k][:, ts(n, NT)],
                            start=(k == 0),
                            stop=False,
                        )

            # Phase 2: per-m tails k=4..7, so group m completes ~8 matmuls
            # after group m-1 and its cast+store overlaps the remaining MMs.
            for m in range(MO):
                for k in range(KO // 2, KO):
                    for n in range(NO):
                        nc.tensor.matmul(
                            pss[(m, n)][:],
                            lhsT=x_sb[k][:, ts(m, P)],
                            rhs=w_sb[k][:, ts(n, NT)],
                            start=False,
                            stop=(k == KO - 1),
                        )
                # n=0 on DVE -> Sync queue; n=1 on ACT -> GpSimd queue
                ot0 = opool.tile([P, NT], f16, tag="ot", name=f"ot{m}_0")
                nc.vector.tensor_copy(out=ot0[:], in_=pss[(m, 0)][:])
                nc.sync.dma_start(out[ts(m, P), :NT], ot0[:])
                ot1 = opool.tile([P, NT], f16, tag="ot", name=f"ot{m}_1")
                nc.scalar.activation(
                    ot1[:],
                    pss[(m, 1)][:],
                    mybir.ActivationFunctionType.Copy,
                )
                nc.gpsimd.dma_start(out[ts(m, P), NT:], ot1[:])
    nc.finalize()
    return nc


def kernel(x, category_id, weight, bias):
    global LAST_RESULTS
    x = np.asarray(x)
    category_id = np.asarray(category_id)
    weight = np.ascontiguousarray(np.asarray(weight), dtype=np.float32)
    bias = np.ascontiguousarray(np.asarray(bias), dtype=np.float32)

    orig_shape = x.shape
    d = orig_shape[-1]
    C, _, o = weight.shape
    assert C == N_CORES and d == D and o == O

    T = int(np.prod(orig_shape[:-1]))
    x_flat = np.ascontiguousarray(x.reshape(T, D), dtype=np.float32)
    cid = category_id.reshape(T).astype(np.int64)

    idx_per_c = [np.flatnonzero(cid == c) for c in range(C)]
    dev_idx = [ix[:CAP] for ix in idx_per_c]
    over_idx = [ix[CAP:] for ix in idx_per_c]

    if "nc" not in _nc_cache:
        _nc_cache["nc"] = _build_nc()
    nc = _nc_cache["nc"]

    in_maps = []
    for c in range(C):
        xcT = np.zeros((D, CAP), dtype=np.float16)
        n = len(dev_idx[c])
        xcT[:, :n] = x_flat[dev_idx[c]].astype(np.float16).T
        in_maps.append({"xT": xcT, "w": weight[c].astype(np.float16)})

    res = run_bass_kernel_spmd(nc, in_maps, list(range(N_CORES)))
    LAST_RESULTS = res

    out_flat = np.empty((T, O), dtype=np.float32)
    for c in range(C):
        n = len(dev_idx[c])
        out_flat[dev_idx[c]] = res.results[c]["out"][:n].astype(np.float32) + bias[c]
        if len(over_idx[c]):
            # capacity overflow (counts are ~512±25; a handful of tokens):
            # exact fp32 on host as part of the unshard/scatter step
            out_flat[over_idx[c]] = x_flat[over_idx[c]] @ weight[c] + bias[c]
    return out_flat.reshape(*orig_shape[:-1], O)


# revision 11
# speedup vs baseline: 1.0065x; 1.0065x over previous
"""CategorySpecificLinear Trainium2 kernel.

out[t] = x[t] @ weight[category_id[t]] + bias[category_id[t]]

Strategy: expert-parallel over the 8 categories (C == n_cores == 8) with a
fixed device capacity of CAP=512 tokens per core. Host routes tokens by
category; the few tokens beyond 512 in an over-subscribed category (counts
are ~512 +/- 25 for T=4096 uniform tokens) are computed on the host during
the unshard step, so the NEFF shape is static.

All device traffic is fp16 (tolerance is 2e-2; fp16 in/out measures ~4e-4):
    xT  [D=1024, 512]  tokens of category c, transposed, zero-padded
    w   [D, O]         weight[c]
    out [512, O]       fp16; bias (+ fp32 cast) is folded into the host
                       scatter -- a vectorized add during unsharding.

Compute is x-stationary: psum[m,n] (+)= x[k,m].T @ w[k,n] over k, with
m = 4 token-tiles of 128 and n = 2 O-halves of 512 -- exactly the 8 fp32
PSUM banks. MM order is phase1 k=0..3 for all (m,n) (gated only on the
k-major DMA stream), then per-m tails k=4..7, so the 8 psum groups COMPLETE
staggered ~2 us apart and the psum->sbuf cast + store of group i overlaps
the matmuls of groups i+1.. instead of stacking after the last MM. Each
LDWEIGHTS x[k,m] is shared by the n-pair of matmuls.

Loads stream k-major in 128 KB pieces (x[k], w[k] in halves) round-robined
over both HWDGE queues (the pair saturates the ~358 GB/s per-core HBM
limit); casts split DVE/ACT; stores split Sync/GpSimd queues.

Per-core HBM traffic ~4.2 MB; PE stream is 64 matmuls @ N=512 fp16.
"""

import contextlib
import ctypes
import os
import sys
import types

import numpy as np

sys.path.insert(0, "/opt/trn_rl_repo")


def _ensure_ntff_hook():
    """Provide antenv.axon_hooks if the image lacks it.

    concourse.bass_utils imports antenv.axon_hooks.get_axon_ntff_profile_hook
    when trace=True under axon; some agent images don't ship that module, in
    which case the boot's NTFF hook registration silently degrades and the
    import in bass_utils crashes. Recreate the slim ctypes hook here
    (mirrors trn_agent_boot.trn_boot._ntff_profile_via_ctypes).
    """
    try:
        import antenv.axon_hooks  # noqa: F401

        return
    except ImportError:
        pass

    so_path = "/opt/axon/libaxon_pjrt.so"
    hook = None
    if os.path.exists(so_path):
        lib = ctypes.CDLL(so_path)
        if hasattr(lib, "axon_start_nrt_profile"):
            lib.axon_start_nrt_profile.argtypes = [
                ctypes.POINTER(ctypes.c_int64),
                ctypes.c_size_t,
            ]
            lib.axon_start_nrt_profile.restype = ctypes.c_int64
            lib.axon_stop_nrt_profile.argtypes = [ctypes.c_char_p]
            lib.axon_stop_nrt_profile.restype = ctypes.c_int64

            @contextlib.contextmanager
            def hook(output_dir, device_ids):
                import jax

                jax.devices()
                if device_ids:
                    ids = (ctypes.c_int64 * len(device_ids))(*device_ids)
                    rc = lib.axon_start_nrt_profile(ids, len(device_ids))
                else:
                    rc = lib.axon_start_nrt_profile(None, 0)
                if rc != 0:
                    raise RuntimeError(f"axon_start_nrt_profile rc={rc}")
                try:
                    yield
                finally:
                    n = lib.axon_stop_nrt_profile(str(output_dir).encode())
                    if n <= 0:
                        print(
                            f"ntff profile: rc={n} writing {output_dir}",
                            file=sys.stderr,
                        )

    mod = types.ModuleType("antenv.axon_hooks")
    _state = {"hook": hook}
    mod.set_axon_ntff_profile_hook = lambda h: _state.__setitem__("hook", h)
    mod.get_axon_ntff_profile_hook = lambda: _state["hook"]
    sys.modules["antenv.axon_hooks"] = mod
    try:
        import antenv

        antenv.axon_hooks = mod
    except ImportError:
        pass


_ensure_ntff_hook()

import concourse.bass as bass
import concourse.bacc as bacc_mod
import concourse.mybir as mybir
import concourse.tile as tile
from concourse.bass import ts
from concourse.bass_utils import run_bass_kernel_spmd

N_CORES = 8
P = 128
CAP = 512  # device tokens per core
D = 1024
O = 1024
KO = D // P  # 8 contraction slices
MO = CAP // P  # 4 token tiles
NT = 512  # O-half (one fp32 PSUM bank)
NO = O // NT  # 2

_nc_cache = {}
LAST_RESULTS = None  # BassKernelResults of the most recent run (for test.py)


def _build_nc():
    f16 = mybir.dt.float16
    f32 = mybir.dt.float32

    nc = bacc_mod.Bacc()
    # partition-major streams (host-packed) so every load chunk has >=2 KB
    # contiguous per-partition lines: xq[p, k*CAP+t] = x[t, k*128+p],
    # wlo[p, k*NT+j] = w[k*128+p, j], whi[p, k*NT+j] = w[k*128+p, NT+j]
    xq = nc.dram_tensor("xq", [P, KO * CAP], f16, kind="ExternalInput")
    wlo = nc.dram_tensor("wlo", [P, KO * NT], f16, kind="ExternalInput")
    whi = nc.dram_tensor("whi", [P, KO * NT], f16, kind="ExternalInput")
    out = nc.dram_tensor("out", [CAP, O], f16, kind="ExternalOutput")

    with tile.TileContext(nc) as tc:
        with (
            tc.tile_pool(name="resident", bufs=1) as rpool,
            tc.tile_pool(name="psum", bufs=8, space="PSUM") as psum_pool,
            tc.tile_pool(name="obuf", bufs=8) as opool,
        ):
            # HAM warm-up: dummy matmuls from when the engine frees (~7 us)
            # until the first k-slice lands (~9.3 us) pull the 3.4 us HAM
            # busy-window forward so the real stream runs at 2.4 GHz sooner.
            # The warm psum tile is the first allocation of the 8-buf "ps"
            # ring; its bank is recycled for the last psum group (warm-up is
            # long done by that group's first MM).
            warm_sb = rpool.tile([P, P], f16, tag="warm")
            nc.vector.memset(warm_sb[:], 0.0)
            warm_ps = psum_pool.tile([P, NT], f32, tag="ps", name="warm_ps")
            for _ in range(16):
                nc.tensor.matmul(
                    warm_ps[:, :P],
                    lhsT=warm_sb[:],
                    rhs=warm_sb[:],
                    start=True,
                    stop=True,
                )

            # Loads on THREE queues in k-ramped chunks [k0],[k1],[k2,k3],
            # [k4,k5],[k6,k7]: x stream on Sync, w-lo on Scalar, w-hi on
            # GpSimd. All three queues contribute to every k-slice, pacing
            # arrivals ~1 us/slice under the shared ~320 GB/s HBM rate, and
            # the pair chunks have 2 KB lines for descriptor efficiency.
            chunks = [(0, 1), (1, 2), (2, 4), (4, 6), (6, 8)]
            x_c, wlo_c, whi_c = [], [], []
            for lo, hi in chunks:
                nk = hi - lo
                xt = rpool.tile([P, nk * CAP], f16, tag=f"x{lo}")
                nc.sync.dma_start(xt[:], xq[:, lo * CAP : hi * CAP])
                x_c.append(xt)
                wl = rpool.tile([P, nk * NT], f16, tag=f"wl{lo}")
                nc.scalar.dma_start(wl[:], wlo[:, lo * NT : hi * NT])
                wlo_c.append(wl)
                wh = rpool.tile([P, nk * NT], f16, tag=f"wh{lo}")
                nc.gpsimd.dma_start(wh[:], whi[:, lo * NT : hi * NT])
                whi_c.append(wh)

            cidx = {}  # k -> (chunk index, offset within chunk)
            for j, (lo, hi) in enumerate(chunks):
                for k in range(lo, hi):
                    cidx[k] = (j, k - lo)

            def x_ap(k, m):
                j, off = cidx[k]
                return x_c[j][:, off * CAP + m * P : off * CAP + (m + 1) * P]

            def w_ap(k, n):
                j, off = cidx[k]
                c = wlo_c if n == 0 else whi_c
                return c[j][:, off * NT : (off + 1) * NT]

            pss = {
                (m, n): psum_pool.tile([P, NT], f32, tag="ps", name=f"ps{m}_{n}")
                for m in range(MO)
                for n in range(NO)
            }

            # Phase 1: k=0..3 for every (m,n) -- rides the DMA stream.
            # n-outer so the n=0 matmuls never wait for the w-hi piece
            # (which lands on the slower-latency GpSimd queue).
            for k in range(KO // 2):
                for n in range(NO):
                    for m in range(MO):
                        nc.tensor.matmul(
                            pss[(m, n)][:],
                            lhsT=x_ap(k, m),
                            rhs=w_ap(k, n),
                            start=(k == 0),
                            stop=False,
                        )

            # Phase 2: per-m tails k=4..7, so group m completes ~8 matmuls
            # after group m-1 and its cast+store overlaps the remaining MMs.
            for m in range(MO):
                for k in range(KO // 2, KO):
                    for n in range(NO):
                        nc.tensor.matmul(
                            pss[(m, n)][:],
                            lhsT=x_ap(k, m),
                            rhs=w_ap(k, n),
                            start=False,
                            stop=(k == KO - 1),
                        )
                # n=0 on DVE -> Sync queue; n=1 on ACT -> GpSimd queue
                ot0 = opool.tile([P, NT], f16, tag="ot", name=f"ot{m}_0")
                nc.vector.tensor_copy(out=ot0[:], in_=pss[(m, 0)][:])
                nc.sync.dma_start(out[ts(m, P), :NT], ot0[:])
                ot1 = opool.tile([P, NT], f16, tag="ot", name=f"ot{m}_1")
                nc.scalar.activation(
                    ot1[:],
                    pss[(m, 1)][:],
                    mybir.ActivationFunctionType.Copy,
                )
                nc.gpsimd.dma_start(out[ts(m, P), NT:], ot1[:])
    nc.finalize()
    return nc


def kernel(x, category_id, weight, bias):
    global LAST_RESULTS
    x = np.asarray(x)
    category_id = np.asarray(category_id)
    weight = np.ascontiguousarray(np.asarray(weight), dtype=np.float32)
    bias = np.ascontiguousarray(np.asarray(bias), dtype=np.float32)

    orig_shape = x.shape
    d = orig_shape[-1]
    C, _, o = weight.shape
    assert C == N_CORES and d == D and o == O

    T = int(np.prod(orig_shape[:-1]))
    x_flat = np.ascontiguousarray(x.reshape(T, D), dtype=np.float32)
    cid = category_id.reshape(T).astype(np.int64)

    idx_per_c = [np.flatnonzero(cid == c) for c in range(C)]
    dev_idx = [ix[:CAP] for ix in idx_per_c]
    over_idx = [ix[CAP:] for ix in idx_per_c]

    if "nc" not in _nc_cache:
        _nc_cache["nc"] = _build_nc()
    nc = _nc_cache["nc"]

    in_maps = []
    for c in range(C):
        xcT = np.zeros((D, CAP), dtype=np.float16)
        n = len(dev_idx[c])
        xcT[:, :n] = x_flat[dev_idx[c]].astype(np.float16).T
        # partition-major streams: xq[p, k*CAP+t] = xcT[k*128+p, t], etc.
        xq = np.ascontiguousarray(
            xcT.reshape(KO, P, CAP).transpose(1, 0, 2).reshape(P, KO * CAP)
        )
        w16 = weight[c].astype(np.float16).reshape(KO, P, O)
        wlo = np.ascontiguousarray(
            w16[:, :, :NT].transpose(1, 0, 2).reshape(P, KO * NT)
        )
        whi = np.ascontiguousarray(
            w16[:, :, NT:].transpose(1, 0, 2).reshape(P, KO * NT)
        )
        in_maps.append({"xq": xq, "wlo": wlo, "whi": whi})

    res = run_bass_kernel_spmd(nc, in_maps, list(range(N_CORES)))
    LAST_RESULTS = res

    out_flat = np.empty((T, O), dtype=np.float32)
    for c in range(C):
        n = len(dev_idx[c])
        out_flat[dev_idx[c]] = res.results[c]["out"][:n].astype(np.float32) + bias[c]
        if len(over_idx[c]):
            # capacity overflow (counts are ~512±25; a handful of tokens):
            # exact fp32 on host as part of the unshard/scatter step
            out_flat[over_idx[c]] = x_flat[over_idx[c]] @ weight[c] + bias[c]
    return out_flat.reshape(*orig_shape[:-1], O)
